# revision 17
# baseline (speedup 1.0000x reference)
# Trainium2 Bass kernel for CriticRNN: embed -> GRU scan (done-reset) -> critic MLP.
#
# Sharding: data-parallel over batch B=256 across 8 NeuronCores (32 envs/core).
# Weights replicated; the T=512 time scan runs locally per shard. No collectives;
# host scatters inputs / gathers outputs.
#
# Per-core device program (one TileContext):
#   G1+G2 (fused): embT = relu(W_emb^T obsT + b_emb)   [D, R]   (obs transposed on PE)
#                  xgT  = Wi^T embT + bi               [3D, R]  -> DRAM scratch
#   SCAN: for t: hg = Wh^T h (d-major, bf16 MMs); gates on DVE/ACT; ysT col t -> DRAM
#   G4+G5 (fused): critic = relu(W1^T ysT + b1); value = W2^T critic + b2 -> [1, R]
#
# Matmuls run as float32r (full-rate when the moving free dim >= 256); the scan's
# recurrent matmul uses bf16 weights/hidden (moving dim is only 32).

import numpy as np
import ml_dtypes

import concourse.bass as bass
from concourse import bacc
import concourse.mybir as mybir
import concourse.tile as tile
from concourse.bass import ds
from concourse.bass_utils import run_bass_kernel_spmd
from concourse.masks import make_identity

T, B, OBS, D, H = 512, 256, 256, 512, 512
NCORES = 8
BS = B // NCORES            # 32 envs per core
R = T * BS                  # 16384 rows (t-major)
G3 = 3 * D                  # 1536
U = 16                      # scan steps per For_i iteration
C = T // U                  # 32 iterations
NCH = R // 512              # 32 row-chunks of 512 for the GEMM phases

F32 = mybir.dt.float32
F32R = mybir.dt.float32r
BF16 = mybir.dt.bfloat16
FP16 = mybir.dt.float16
AF = mybir.ActivationFunctionType
OP = mybir.AluOpType


def _mm(nc, out, lhsT, rhs, start, stop):
    nc.tensor.matmul(out, lhsT, rhs, start=start, stop=stop)


def build_nc():
    nc = bacc.Bacc(trn_type="TRN2", target_bir_lowering=False, debug=False)

    # ---- I/O ----
    obs = nc.dram_tensor("obs", [R, OBS], F32, kind="ExternalInput")
    xmask = nc.dram_tensor("xmask", [T, 128], F32, kind="ExternalInput")
    h0T = nc.dram_tensor("h0T", [128, 128], F32, kind="ExternalInput")
    W_emb = nc.dram_tensor("W_emb", [OBS, D], FP16, kind="ExternalInput")
    b_emb = nc.dram_tensor("b_emb", [D], F32, kind="ExternalInput")
    Wi = nc.dram_tensor("Wi", [D, G3], FP16, kind="ExternalInput")
    bi = nc.dram_tensor("bi", [G3], F32, kind="ExternalInput")
    Whb = nc.dram_tensor("Whb", [D, G3], FP16, kind="ExternalInput")
    bhn_ext = nc.dram_tensor("bhn_ext", [128, 128], F32, kind="ExternalInput")
    W1 = nc.dram_tensor("W1", [D, H], FP16, kind="ExternalInput")
    b1 = nc.dram_tensor("b1", [H], F32, kind="ExternalInput")
    W2 = nc.dram_tensor("W2", [H, 1], FP16, kind="ExternalInput")
    b2 = nc.dram_tensor("b2", [1, 1], F32, kind="ExternalInput")

    h_lastT = nc.dram_tensor("h_lastT", [128, 128], F32, kind="ExternalOutput")
    value = nc.dram_tensor("value", [1, R], F32, kind="ExternalOutput")

    # scan-native scratch layouts: [chunk, partition, step, block, batch]
    xgT = nc.dram_tensor("xgT", [C, 128, U, 12, BS], F32)   # xg, d-major-chunked
    ysT = nc.dram_tensor("ysT", [C, 128, U, 4, BS], F32)    # ys, d-major-chunked

    with tile.TileContext(nc) as tc:
        _build_body(nc, tc, locals())
    nc.compile()
    return nc


def _build_body(nc, tc, t_):
    obs, xmask, h0T = t_["obs"], t_["xmask"], t_["h0T"]
    W_emb, b_emb, Wi, bi = t_["W_emb"], t_["b_emb"], t_["Wi"], t_["bi"]
    Whb, bhn_ext, W1, b1, W2, b2 = (
        t_["Whb"], t_["bhn_ext"], t_["W1"], t_["b1"], t_["W2"], t_["b2"])
    h_lastT, value, xgT, ysT = t_["h_lastT"], t_["value"], t_["xgT"], t_["ysT"]

    from contextlib import ExitStack
    ctx = ExitStack()
    with ctx:
        const = ctx.enter_context(tc.tile_pool(name="const", bufs=1))

        # ---- resident weights / constants in SBUF ----
        wemb_sb = const.tile([128, 2, D], FP16)
        nc.sync.dma_start(wemb_sb[:], W_emb.ap().rearrange("(c p) d -> p c d", p=128))
        bemb_sb = const.tile([128, 4], F32)
        nc.sync.dma_start(bemb_sb[:], b_emb.ap().rearrange("(m p) -> p m", p=128))
        wi_sb = const.tile([128, 4, G3], FP16)
        nc.sync.dma_start(wi_sb[:], Wi.ap().rearrange("(k p) e -> p k e", p=128))
        bi_sb = const.tile([128, 12], F32)
        nc.sync.dma_start(bi_sb[:], bi.ap().rearrange("(m p) -> p m", p=128))
        wh_sb = const.tile([128, 4, G3], FP16)
        nc.sync.dma_start(wh_sb[:], Whb.ap().rearrange("(k p) e -> p k e", p=128))
        bhn_sb = const.tile([128, 128], F32)
        nc.sync.dma_start(bhn_sb[:], bhn_ext.ap())
        w1_sb = const.tile([128, 4, H], FP16)
        nc.sync.dma_start(w1_sb[:], W1.ap().rearrange("(k p) h -> p k h", p=128))
        b1_sb = const.tile([128, 4], F32)
        nc.sync.dma_start(b1_sb[:], b1.ap().rearrange("(m p) -> p m", p=128))
        w2_sb = const.tile([128, 4], FP16)
        nc.sync.dma_start(w2_sb[:], W2.ap().rearrange("(k p) o -> p (k o)", p=128))
        b2_sb = const.tile([1, 1], F32)
        nc.sync.dma_start(b2_sb[:], b2.ap())
        ident = const.tile([128, 128], F32)
        make_identity(nc, ident[:])

        # persistent hidden-state ping-pong (d-major-chunked [128, (k4)(b32)])
        h_a = const.tile([128, 128], F32)
        h_b = const.tile([128, 128], F32)
        hbf_a = const.tile([128, 128], FP16)
        hbf_b = const.tile([128, 128], FP16)
        nc.sync.dma_start(h_a[:], h0T.ap())
        nc.vector.tensor_copy(hbf_a[:], h_a[:])

        # ================= G1+G2: embT then xgT =================
        with tc.tile_pool(name="g12_obs", bufs=3) as obs_pool, \
             tc.tile_pool(name="g12_obst", bufs=2) as obst_pool, \
             tc.tile_pool(name="g12_emb", bufs=8) as emb_pool, \
             tc.tile_pool(name="g12_xo", bufs=3) as xo_pool, \
             tc.tile_pool(name="g12_ptr", bufs=2, space="PSUM") as ptr_pool, \
             tc.tile_pool(name="g12_pe", bufs=2, space="PSUM") as pe_pool, \
             tc.tile_pool(name="g12_p2", bufs=3, space="PSUM") as p2_pool:
            for n in range(NCH):
                obst = obst_pool.tile([128, 2, 512], FP16)
                for q in range(4):
                    ob = obs_pool.tile([128, OBS], F32)
                    nc.sync.dma_start(ob[:], obs.ap()[n * 512 + q * 128:
                                                      n * 512 + (q + 1) * 128, :])
                    pt = ptr_pool.tile([128, 256], F32)
                    nc.tensor.transpose(pt[:, 0:128], ob[:, 0:128], ident[:])
                    nc.tensor.transpose(pt[:, 128:256], ob[:, 128:256], ident[:])
                    nc.vector.tensor_copy(obst[:, 0, q * 128:(q + 1) * 128],
                                          pt[:, 0:128])
                    nc.vector.tensor_copy(obst[:, 1, q * 128:(q + 1) * 128],
                                          pt[:, 128:256])
                embt = []
                for m in range(4):
                    pe = pe_pool.tile([128, 512], F32)
                    for c in range(2):
                        _mm(nc, pe[:], wemb_sb[:, c, m * 128:(m + 1) * 128],
                            obst[:, c, :], start=(c == 0), stop=(c == 1))
                    et = emb_pool.tile([128, 512], FP16)
                    nc.scalar.activation(et[:], pe[:], AF.Relu,
                                         bias=bemb_sb[:, m:m + 1])
                    embt.append(et)
                for m2 in range(12):
                    p2 = p2_pool.tile([128, 512], F32)
                    for k in range(4):
                        _mm(nc, p2[:], wi_sb[:, k, m2 * 128:(m2 + 1) * 128],
                            embt[k][:], start=(k == 0), stop=(k == 3))
                    xo = xo_pool.tile([128, 512], F32)
                    nc.scalar.activation(xo[:], p2[:], AF.Identity,
                                         bias=bi_sb[:, m2:m2 + 1])
                    # rows chunk n == scan chunk c; xo free = (u, b)
                    nc.sync.dma_start(
                        xgT.ap()[n, :, :, m2, :],
                        xo[:].rearrange("p (u b) -> p u b", u=U, b=BS))

        # ================= SCAN =================
        mask_src = xmask.ap().rearrange("(c u) m -> c u m", c=C, u=U)

        with tc.tile_pool(name="sc_xg", bufs=2) as xg_pool, \
             tc.tile_pool(name="sc_ys", bufs=2) as ys_pool, \
             tc.tile_pool(name="sc_mask", bufs=2) as mk_pool, \
             tc.tile_pool(name="sc_tmp", bufs=4) as tmp_pool, \
             tc.tile_pool(name="sc_prz", bufs=2, space="PSUM") as prz_pool, \
             tc.tile_pool(name="sc_pn", bufs=2, space="PSUM") as pn_pool:
            with tc.For_i(0, C, 1) as it:
                xg_sb = xg_pool.tile([128, U * 12 * BS], F32)
                nc.sync.dma_start(
                    xg_sb[:],
                    xgT.ap()[ds(it, 1), :, :, :, :].rearrange(
                        "c p u j b -> p (c u j b)"))
                # mask rows replicated across all 128 partitions by the DMA
                mk_sb = mk_pool.tile([128, U * 128], F32)
                nc.sync.dma_start(
                    mk_sb[:],
                    mask_src[ds(it, 1), :, :].rearrange(
                        "c u m -> (c u) m").partition_broadcast(128))
                ys_sb = ys_pool.tile([128, U * 128], F32)

                for u in range(U):
                    h_in, hbf_in = (h_a, hbf_a) if u % 2 == 0 else (h_b, hbf_b)
                    h_out, hbf_out = (h_b, hbf_b) if u % 2 == 0 else (h_a, hbf_a)

                    prz = prz_pool.tile([128, 256], F32)
                    pn = pn_pool.tile([128, 128], F32)
                    for j in range(12):
                        pslice = (prz[:, j * 32:(j + 1) * 32] if j < 8
                                  else pn[:, (j - 8) * 32:(j - 7) * 32])
                        for k in range(4):
                            nc.tensor.matmul(
                                pslice,
                                wh_sb[:, k, j * 128:(j + 1) * 128],
                                hbf_in[:, k * 32:(k + 1) * 32],
                                start=(k == 0), stop=(k == 3))

                    xg_u = xg_sb[:, u * 384:(u + 1) * 384]
                    rz = tmp_pool.tile([128, 256], F32, tag="rz")
                    nc.vector.tensor_tensor(rz[:], prz[:], xg_u[:, 0:256], OP.add)
                    rzs = tmp_pool.tile([128, 256], F32, tag="rzs")
                    nc.scalar.activation(rzs[:], rz[:], AF.Sigmoid)
                    nb = tmp_pool.tile([128, 128], F32, tag="nb")
                    nc.vector.tensor_tensor(nb[:], pn[:], bhn_sb[:], OP.add)
                    t1 = tmp_pool.tile([128, 128], F32, tag="t1")
                    nc.vector.tensor_tensor(t1[:], nb[:], rzs[:, 0:128], OP.mult)
                    t2 = tmp_pool.tile([128, 128], F32, tag="t2")
                    nc.vector.tensor_tensor(t2[:], t1[:], xg_u[:, 256:384], OP.add)
                    sg = tmp_pool.tile([128, 128], F32, tag="sg")
                    nc.scalar.activation(sg[:], t2[:], AF.Sigmoid, scale=2.0)
                    ng = tmp_pool.tile([128, 128], F32, tag="ng")
                    nc.vector.tensor_scalar(ng[:], sg[:], 2.0, 1.0,
                                            OP.mult, OP.subtract)
                    t3 = tmp_pool.tile([128, 128], F32, tag="t3")
                    nc.vector.tensor_tensor(t3[:], h_in[:], ng[:], OP.subtract)
                    t4 = tmp_pool.tile([128, 128], F32, tag="t4")
                    nc.vector.tensor_tensor(t4[:], rzs[:, 128:256], t3[:], OP.mult)
                    ys_u = ys_sb[:, u * 128:(u + 1) * 128]
                    nc.vector.tensor_tensor(ys_u, t4[:], ng[:], OP.add)
                    mrow = mk_sb[:, u * 128:(u + 1) * 128]
                    nc.vector.tensor_tensor(h_out[:], ys_u, mrow, OP.mult)
                    nc.vector.tensor_copy(hbf_out[:], h_out[:])

                nc.sync.dma_start(
                    ysT.ap()[ds(it, 1), :, :, :, :].rearrange(
                        "c p u k b -> p (c u k b)"),
                    ys_sb[:])

        nc.sync.dma_start(h_lastT.ap(), h_a[:])

        # ================= G4+G5: critic + value =================
        with tc.tile_pool(name="g45_ys", bufs=3) as ysin_pool, \
             tc.tile_pool(name="g45_cr", bufs=3) as cr_pool, \
             tc.tile_pool(name="g45_val", bufs=2) as val_pool, \
             tc.tile_pool(name="g45_p4", bufs=2, space="PSUM") as p4_pool, \
             tc.tile_pool(name="g45_pv", bufs=2, space="PSUM") as pv_pool:
            for n in range(NCH):
                # same layout as the scan wrote: [p, (u k b)] — contiguous DMA
                ys_in = ysin_pool.tile([128, U * 4 * BS], F32)
                nc.sync.dma_start(
                    ys_in[:],
                    ysT.ap()[n, :, :, :, :].rearrange("p u k b -> p (u k b)"))
                ys16 = ysin_pool.tile([128, U * 4 * BS], FP16, tag="ys16")
                nc.vector.tensor_copy(ys16[:], ys_in[:])
                ys_k = ys16[:].rearrange("p (u k b) -> p k u b", u=U, k=4, b=BS)
                pv = pv_pool.tile([1, 512], F32)
                for m in range(4):
                    p4 = p4_pool.tile([128, 512], F32)
                    for k in range(4):
                        _mm(nc, p4[:], w1_sb[:, k, m * 128:(m + 1) * 128],
                            ys_k[:, k, :, :], start=(k == 0), stop=(k == 3))
                    cr = cr_pool.tile([128, 512], FP16)
                    nc.scalar.activation(cr[:], p4[:], AF.Relu,
                                         bias=b1_sb[:, m:m + 1])
                    _mm(nc, pv[:], w2_sb[:, m:m + 1], cr[:],
                        start=(m == 0), stop=(m == 3))
                val = val_pool.tile([1, 512], F32)
                nc.scalar.activation(val[:], pv[:], AF.Identity, bias=b2_sb[:])
                nc.sync.dma_start(value.ap()[:, n * 512:(n + 1) * 512], val[:])


_NC_CACHE = None


def _get_nc():
    global _NC_CACHE
    if _NC_CACHE is None:
        _NC_CACHE = build_nc()
    return _NC_CACHE


def _prep_core_inputs(core, obs, done, hstate, W_emb, b_emb, Wi, bi, Wh, bhn,
                      W1, b1, W2, b2):
    b0 = core * BS
    obs_s = np.ascontiguousarray(
        obs[:, b0:b0 + BS, :].reshape(R, OBS).astype(np.float32))
    done_s = done[:, b0:b0 + BS].astype(np.float32)        # [T, BS]
    # xmask row t = mask applied AFTER step t (resets h where done[t+1]); row T-1 = 1
    xm = np.ones((T, BS), np.float32)
    xm[:T - 1] = 1.0 - done_s[1:]
    xmask = np.ascontiguousarray(np.tile(xm, (1, 4)))      # [T, 128] (k-blocked b)
    # initial h, pre-masked with done[0], d-major-chunked [128, (k4)(b32)]
    h0 = hstate[b0:b0 + BS].astype(np.float32) * (1.0 - done_s[0])[:, None]
    h0T = np.ascontiguousarray(
        h0.reshape(BS, 4, 128).transpose(2, 1, 0).reshape(128, 128))
    # bhn_ext[p, jj*32+b] = bhn[jj*128+p]
    bhn_ext = np.ascontiguousarray(
        np.repeat(bhn.astype(np.float32).reshape(4, 128).T[:, :, None],
                  BS, axis=2).reshape(128, 128))
    return {
        "obs": obs_s,
        "xmask": xmask,
        "h0T": h0T,
        "W_emb": np.ascontiguousarray(np.asarray(W_emb).astype(np.float16)),
        "b_emb": np.ascontiguousarray(b_emb.astype(np.float32)),
        "Wi": np.ascontiguousarray(np.asarray(Wi).astype(np.float16)),
        "bi": np.ascontiguousarray(bi.astype(np.float32)),
        "Whb": np.ascontiguousarray(np.asarray(Wh).astype(np.float16)),
        "bhn_ext": bhn_ext,
        "W1": np.ascontiguousarray(np.asarray(W1).astype(np.float16)),
        "b1": np.ascontiguousarray(b1.astype(np.float32)),
        "W2": np.ascontiguousarray(np.asarray(W2).astype(np.float16)),
        "b2": np.ascontiguousarray(b2.astype(np.float32).reshape(1, 1)),
    }


def kernel(obs, done, hstate, W_emb, b_emb, Wi, bi, Wh, bhn, W1, b1, W2, b2):
    obs = np.asarray(obs)
    done = np.asarray(done)
    hstate = np.asarray(hstate)
    nc = _get_nc()
    in_maps = [
        _prep_core_inputs(c, obs, done, hstate, W_emb, b_emb, Wi, bi, Wh, bhn,
                          W1, b1, W2, b2)
        for c in range(NCORES)
    ]
    res = run_bass_kernel_spmd(nc, in_maps, core_ids=list(range(NCORES)))
    h_last = np.empty((B, D), np.float32)
    val = np.empty((T, B), np.float32)
    for c in range(NCORES):
        out = res.results[c]
        hlT = out["h_lastT"]                                # [128, (k4)(b32)]
        h_last[c * BS:(c + 1) * BS] = (
            hlT.reshape(128, 4, BS).transpose(2, 1, 0).reshape(BS, D))
        val[:, c * BS:(c + 1) * BS] = out["value"].reshape(T, BS)
    return h_last, val
